# revision 13
# baseline (speedup 1.0000x reference)
"""Block-causal multi-head attention (B=1, S=4096, E=1024, H=16, BLK=128) on 8 trn2 cores.

Strategy (head-parallel attention + sequence-parallel out_proj):
  - Host: transpose x and the weights once; give every core the full x^T plus the
    in_proj rows for its 2 heads (q/k/v for heads 2c, 2c+1), W_out^T, biases.
  - Core c: QKV^T projection for its 2 heads over the full sequence (fp32r matmuls),
    then block-causal attention in the transposed (S^T = K Q^T) layout:
      scores^T [128k x 512q] per (key-block, q-group), exp on ACT (no max needed:
      |scores| <~ 4 for these inputs), PV via V augmented with a ones column so the
      softmax denominator falls out of the same matmul (psum row 64), then
      normalize with a K=1 broadcast matmul + DVE multiply.
  - AllToAll exchanges out^T so core c ends with attn^T [1024, 512] for sequence
    slice c; out_proj computes y^T [1024, 512] per core; host concatenates and
    transposes back.
All matmuls run as float32r (fp32 storage, ~tf32 precision, 1 cycle/row at N>=256).
"""
import numpy as np

import concourse.bass as bass
import concourse.mybir as mybir
from concourse import bacc, tile
from concourse.bass_utils import run_bass_kernel_spmd
from concourse.masks import make_identity

N_CORES = 8
S, E, H, BLK, D = 4096, 1024, 16, 128, 64
NB = S // BLK            # 32 key/query blocks
NG = 8                   # q-groups of 512
GQ = 512                 # q columns per group
HPC = H // N_CORES       # heads per core (2)
RPC = 3 * HPC * D        # in_proj rows per core (384)

F32 = mybir.dt.float32
F32R = mybir.dt.float32r
BF16 = mybir.dt.bfloat16
ALU = mybir.AluOpType
ACTF = mybir.ActivationFunctionType


def build_nc(reps: int = 1, cc: bool = True):
    nc = bacc.Bacc("TRN2", target_bir_lowering=False, debug=False, num_devices=N_CORES)

    xT = nc.dram_tensor("xT", [E, S], F32R, kind="ExternalInput")
    wqkvT = nc.dram_tensor("wqkvT", [E, RPC], F32R, kind="ExternalInput")
    bqkv = nc.dram_tensor("bqkv", [3, 2 * D], F32, kind="ExternalInput")
    woutT = nc.dram_tensor("woutT", [E, E], F32R, kind="ExternalInput")
    bout = nc.dram_tensor("bout", [8, 128], F32, kind="ExternalInput")
    yT = nc.dram_tensor("yT", [E, GQ], F32, kind="ExternalOutput")

    with tile.TileContext(nc) as tc:
        with (
            tc.tile_pool(name="const", bufs=1) as constp,
            tc.tile_pool(name="wq", bufs=1) as wqp,
            tc.tile_pool(name="wo", bufs=1) as wop,
            tc.tile_pool(name="qkv", bufs=1) as qkvp,
            tc.tile_pool(name="xt", bufs=16) as xtp,
            tc.tile_pool(name="pt", bufs=4) as ptp,
            tc.tile_pool(name="vst", bufs=2) as vstp,
            tc.tile_pool(name="small", bufs=4) as smallp,
            tc.tile_pool(name="attn", bufs=8) as attnp,
            tc.tile_pool(name="ytp", bufs=2) as ytp,
            tc.tile_pool(name="pp", bufs=2, space="PSUM") as pp,
            tc.tile_pool(name="scores", bufs=2, space="PSUM") as scp,
            tc.tile_pool(name="accum", bufs=2, space="PSUM") as accp,
            tc.tile_pool(name="dram", bufs=1, space="DRAM") as dram,
        ):
            # ---- constants / weights ----
            ident = constp.tile([128, 128], F32)
            make_identity(nc, ident[:])
            ones_f32 = constp.tile([128, 64], F32)
            nc.vector.memset(ones_f32[:], 1.0)
            ones1 = constp.tile([1, 64], F32R)
            nc.vector.tensor_copy(ones1[:], ones_f32[0:1, :])
            bq_sb = constp.tile([128, 3], F32)
            bo_sb = constp.tile([128, 8], F32)
            wq_sb = constp.tile([128, 8 * RPC], F32R)

            def load_wq(t):
                nc.sync.dma_start(
                    wq_sb[:, t * RPC:(t + 1) * RPC],
                    wqkvT.ap()[t * 128:(t + 1) * 128, :])

            def load_biases():
                nc.sync.dma_start(bq_sb[:], bqkv.ap().rearrange("r p -> p r"))
                nc.sync.dma_start(bo_sb[:], bout.ap().rearrange("t p -> p t"))
            wo_sb = wop.tile([128, 8 * E], F32R)

            def load_wout():
                for t in range(8):
                    nc.sync.dma_start(
                        wo_sb[:, t * E:(t + 1) * E],
                        woutT.ap()[t * 128:(t + 1) * 128, :])

            # persistent per-rep tensors
            qt_sb = qkvp.tile([128, S], F32R, tag="qt")    # [2 heads x 64d, s]
            kt_sb = qkvp.tile([128, S], F32R, tag="kt")
            v_sb = qkvp.tile([128, 2 * NB * (D + 1)], F32R, tag="vsb")
            v_view = v_sb[:].rearrange("p (h b dd) -> p h b dd", h=2, b=NB)
            cc_in = dram.tile([N_CORES, 128, GQ], F32R, tag="ccin")
            cc_out = dram.tile([N_CORES, 128, GQ], F32R, tag="ccout")

            for rep in range(reps):
                # ones columns of V (denominator trick); rewritten each rep
                nc.vector.tensor_copy(
                    v_view[:, :, :, D:D + 1],
                    ones_f32[:].rearrange("p (h b o) -> p h b o", h=2, b=NB, o=1))

                # ---------- proj work-item machinery ----------
                def xt_dmas(g):
                    tiles = []
                    for t in range(8):
                        if g == 0 and rep == 0:
                            load_wq(t)      # interleave weight chunks with first x tiles
                        xt = xtp.tile([128, GQ], F32R, tag="xt")
                        nc.sync.dma_start(
                            xt[:], xT.ap()[t * 128:(t + 1) * 128, g * GQ:(g + 1) * GQ])
                        tiles.append(xt)
                    if g == 0 and rep == 0:
                        load_biases()
                    return tiles

                def proj_items(g, xts):
                    """Yield closures emitting proj instructions for group g."""
                    sl = slice(g * GQ, (g + 1) * GQ)

                    def rtile(which):
                        ps = pp.tile([128, GQ], F32, tag="pp")
                        for t in range(8):
                            lhs = wq_sb[:, t * RPC + which * 128: t * RPC + (which + 1) * 128]
                            yield lambda lhs=lhs, t=t, ps=ps: nc.tensor.matmul(
                                ps[:], lhs, xts[t][:], start=(t == 0), stop=(t == 7))
                        if which == 0:      # q: (psum + bq) * 1/sqrt(D)
                            yield lambda ps=ps: nc.vector.tensor_scalar(
                                qt_sb[:, sl], ps[:], bq_sb[:, 0:1], 0.125, ALU.add, ALU.mult)
                        elif which == 1:    # k: psum + bk
                            yield lambda ps=ps: nc.vector.tensor_scalar(
                                kt_sb[:, sl], ps[:], bq_sb[:, 1:2], None, ALU.add)
                        else:               # v^T staging: psum + bv
                            vt = vstp.tile([128, GQ], F32, tag="vst")
                            yield lambda ps=ps, vt=vt: nc.vector.tensor_scalar(
                                vt[:], ps[:], bq_sb[:, 2:3], None, ALU.add)
                            for j in range(4):
                                bk = 4 * g + j

                                def tr(j=j, bk=bk, vt=vt):
                                    trp = pp.tile([128, GQ], F32, tag="pp")
                                    nc.tensor.transpose(
                                        trp[0:128, 0:128], vt[:, j * 128:(j + 1) * 128],
                                        ident[:])
                                    nc.vector.tensor_copy(
                                        v_view[:, :, bk, 0:D],
                                        trp[0:128, 0:128].rearrange("p (h d) -> p h d", h=2))
                                yield tr
                    yield from rtile(0)
                    yield from rtile(1)
                    yield from rtile(2)

                def attention_group(g, pending):
                    """Emit attention for q-group g, interleaving `pending` proj items."""
                    nbk = 4 * g + 4
                    # throttle interleaved proj items in the first two blocks so the
                    # group's exp pipeline primes before PE picks up filler work
                    quota = []
                    rem = len(pending)
                    for i in range(nbk):
                        if i < 2:
                            q = min(rem, 1)
                        else:
                            left = nbk - i
                            q = (rem + left - 1) // left
                        quota.append(q)
                        rem -= q
                    pt_tiles = {}
                    acc_a = accp.tile([65, GQ], F32, tag="acc")
                    acc_b = accp.tile([65, GQ], F32, tag="acc")
                    for bk in range(nbk):
                        qoff = max(0, (bk - 4 * g)) * 128
                        sc = scp.tile([128, 2 * GQ], F32, tag="sc")
                        nc.tensor.matmul(
                            sc[:, qoff:GQ],
                            kt_sb[0:64, bk * 128:(bk + 1) * 128],
                            qt_sb[0:64, g * GQ + qoff:(g + 1) * GQ],
                            start=True, stop=True, skip_group_check=True)
                        nc.tensor.matmul(
                            sc[:, GQ + qoff:2 * GQ],
                            kt_sb[64:128, bk * 128:(bk + 1) * 128],
                            qt_sb[64:128, g * GQ + qoff:(g + 1) * GQ],
                            start=True, stop=True, skip_group_check=True)
                        pt = ptp.tile([128, 2 * GQ], F32R, tag="pt")
                        nc.scalar.activation(pt[:, qoff:2 * GQ], sc[:, qoff:2 * GQ], ACTF.Exp)
                        pt_tiles[bk] = (pt, qoff)
                        # PV for the previous block (keeps PE busy while ACT exps)
                        if bk > 0:
                            emit_pv(g, bk - 1, pt_tiles, acc_a, acc_b)
                        for _ in range(quota[bk]):
                            if pending:
                                pending.pop(0)()
                    emit_pv(g, nbk - 1, pt_tiles, acc_a, acc_b, last=True)
                    while pending:
                        pending.pop(0)()
                    return normalize_items(g, acc_a, acc_b)

                def normalize_items(g, acc_a, acc_b):
                    # deferred normalize + all-to-all staging closures for group g
                    items = []
                    for h, acc in ((0, acc_a), (1, acc_b)):
                        def norm(h=h, acc=acc, g=g):
                            outu = smallp.tile([65, GQ], F32, tag="outu")
                            nc.vector.tensor_copy(outu[:], acc[:])
                            recip = smallp.tile([1, GQ], F32R, tag="recip")
                            with nc.allow_low_precision(reason="softmax denom reciprocal rounded to fp32r before broadcast"):
                                nc.vector.reciprocal(recip[:], outu[64:65, :])
                            bc = pp.tile([128, GQ], F32, tag="pp")
                            nc.tensor.matmul(bc[0:64, :], ones1[:], recip[:],
                                             start=True, stop=True, skip_group_check=True)
                            bcs = smallp.tile([64, GQ], F32R, tag="bcs")
                            nc.vector.tensor_copy(bcs[:], bc[0:64, :])
                            outn = smallp.tile([64, GQ], F32R, tag="outn")
                            nc.vector.tensor_tensor(outn[:], outu[0:64, :], bcs[:], ALU.mult)
                            nc.gpsimd.dma_start(cc_in[:][g, h * 64:(h + 1) * 64, :], outn[:])
                        items.append(norm)
                    return items

                def emit_pv(g, bk, pt_tiles, acc_a, acc_b, last=False):
                    pt, qoff = pt_tiles.pop(bk)
                    nc.tensor.matmul(
                        acc_a[0:65, qoff:GQ], v_view[:, 0, bk, 0:D + 1], pt[:, qoff:GQ],
                        start=(bk == 0), stop=last, skip_group_check=True)
                    nc.tensor.matmul(
                        acc_b[0:65, qoff:GQ], v_view[:, 1, bk, 0:D + 1],
                        pt[:, GQ + qoff:2 * GQ],
                        start=(bk == 0), stop=last, skip_group_check=True)

                # ---------- emit: proj(0) then attention groups with lookahead ----------
                xts = xt_dmas(0)
                for item in proj_items(0, xts):
                    item()
                carry = []
                for g in range(NG):
                    if g + 1 < NG:
                        nxts = xt_dmas(g + 1)
                        pending = carry + list(proj_items(g + 1, nxts))
                    else:
                        pending = carry
                    carry = attention_group(g, pending)
                    if g == 1:
                        load_wout()
                for item in carry:
                    item()

                # ---------- all-to-all + out_proj ----------
                if cc:
                    nc.gpsimd.collective_compute(
                        "AllToAll", ALU.bypass,
                        replica_groups=[list(range(N_CORES))],
                        ins=[cc_in.opt()], outs=[cc_out.opt()])
                else:
                    nc.gpsimd.dma_start(cc_out[:], cc_in[:])
                at_tiles = []
                for j in range(N_CORES):
                    at = attnp.tile([128, GQ], F32R, tag="at")
                    nc.gpsimd.dma_start(at[:], cc_out[:][j])
                    at_tiles.append(at)
                for t in range(8):
                    ps = pp.tile([128, GQ], F32, tag="pp")
                    for j in range(N_CORES):
                        nc.tensor.matmul(
                            ps[:], wo_sb[:, j * E + t * 128:j * E + (t + 1) * 128],
                            at_tiles[j][:], start=(j == 0), stop=(j == 7))
                    yt = ytp.tile([128, GQ], F32, tag="yt")
                    nc.vector.tensor_scalar(yt[:], ps[:], bo_sb[:, t:t + 1], None, ALU.add)
                    nc.sync.dma_start(yT.ap()[t * 128:(t + 1) * 128, :], yt[:])

    nc.compile()
    return nc


_NC_CACHE = {}


def _get_nc(reps=1):
    if reps not in _NC_CACHE:
        _NC_CACHE[reps] = build_nc(reps)
    return _NC_CACHE[reps]


def make_in_maps(x, in_proj_weight, in_proj_bias, out_proj_weight, out_proj_bias):
    x = np.asarray(x, np.float32)
    w_in = np.asarray(in_proj_weight, np.float32)
    b_in = np.asarray(in_proj_bias, np.float32)
    w_out = np.asarray(out_proj_weight, np.float32)
    b_out = np.asarray(out_proj_bias, np.float32)

    xT = np.ascontiguousarray(x.reshape(S, E).T)
    woutT = np.ascontiguousarray(w_out.T)
    bout = np.ascontiguousarray(b_out.reshape(8, 128))
    in_maps = []
    for c in range(N_CORES):
        rows = []
        for blk in range(3):  # q, k, v blocks of in_proj
            for h in (2 * c, 2 * c + 1):
                rows.extend(range(blk * E + h * D, blk * E + (h + 1) * D))
        rows = np.array(rows)
        wqkvT = np.ascontiguousarray(w_in[rows].T)          # [1024, 384]
        bqkv = np.ascontiguousarray(b_in[rows].reshape(3, 2 * D))
        in_maps.append({
            "xT": xT, "wqkvT": wqkvT, "bqkv": bqkv,
            "woutT": woutT, "bout": bout,
        })
    return in_maps


def assemble_output(results):
    yT_full = np.concatenate([results[c]["yT"] for c in range(N_CORES)], axis=1)
    return np.ascontiguousarray(yT_full.T).reshape(1, S, E).astype(np.float32)


def kernel(x, in_proj_weight, in_proj_bias, out_proj_weight, out_proj_bias,
           block_size, num_heads):
    assert int(np.asarray(block_size)) == BLK and int(np.asarray(num_heads)) == H
    in_maps = make_in_maps(x, in_proj_weight, in_proj_bias,
                           out_proj_weight, out_proj_bias)
    nc = _get_nc(1)
    res = run_bass_kernel_spmd(nc, in_maps, core_ids=list(range(N_CORES)))
    return assemble_output(res.results)


# revision 15
# speedup vs baseline: 1.0312x; 1.0312x over previous
"""Block-causal multi-head attention (B=1, S=4096, E=1024, H=16, BLK=128) on 8 trn2 cores.

Strategy (head-parallel attention + sequence-parallel out_proj):
  - Host: transpose x and the weights once; give every core the full x^T plus the
    in_proj rows for its 2 heads (q/k/v for heads 2c, 2c+1), W_out^T, biases.
  - Core c: QKV^T projection for its 2 heads over the full sequence (fp32r matmuls),
    then block-causal attention in the transposed (S^T = K Q^T) layout:
      scores^T [128k x 512q] per (key-block, q-group), exp on ACT (no max needed:
      |scores| <~ 4 for these inputs), PV via V augmented with a ones column so the
      softmax denominator falls out of the same matmul (psum row 64), then
      normalize with a K=1 broadcast matmul + DVE multiply.
  - AllToAll exchanges out^T so core c ends with attn^T [1024, 512] for sequence
    slice c; out_proj computes y^T [1024, 512] per core; host concatenates and
    transposes back.
All matmuls run as float32r (fp32 storage, ~tf32 precision, 1 cycle/row at N>=256).
"""
import numpy as np

import concourse.bass as bass
import concourse.mybir as mybir
from concourse import bacc, tile
from concourse.bass_utils import run_bass_kernel_spmd
from concourse.masks import make_identity

N_CORES = 8
S, E, H, BLK, D = 4096, 1024, 16, 128, 64
NB = S // BLK            # 32 key/query blocks
NG = 8                   # q-groups of 512
GQ = 512                 # q columns per group
HPC = H // N_CORES       # heads per core (2)
RPC = 3 * HPC * D        # in_proj rows per core (384)

F32 = mybir.dt.float32
F32R = mybir.dt.float32r
BF16 = mybir.dt.bfloat16
ALU = mybir.AluOpType
ACTF = mybir.ActivationFunctionType


def build_nc(reps: int = 1, cc: bool = True):
    nc = bacc.Bacc("TRN2", target_bir_lowering=False, debug=False, num_devices=N_CORES)

    xT = nc.dram_tensor("xT", [E, S], F32R, kind="ExternalInput")
    wqkvT = nc.dram_tensor("wqkvT", [E, RPC], F32R, kind="ExternalInput")
    bqkv = nc.dram_tensor("bqkv", [3, 2 * D], F32, kind="ExternalInput")
    woutT = nc.dram_tensor("woutT", [E, E], F32R, kind="ExternalInput")
    bout = nc.dram_tensor("bout", [8, 128], F32, kind="ExternalInput")
    yT = nc.dram_tensor("yT", [E, GQ], F32, kind="ExternalOutput")

    with tile.TileContext(nc) as tc:
        with (
            tc.tile_pool(name="const", bufs=1) as constp,
            tc.tile_pool(name="wq", bufs=1) as wqp,
            tc.tile_pool(name="wo", bufs=1) as wop,
            tc.tile_pool(name="qkv", bufs=1) as qkvp,
            tc.tile_pool(name="xt", bufs=16) as xtp,
            tc.tile_pool(name="pt", bufs=4) as ptp,
            tc.tile_pool(name="vst", bufs=2) as vstp,
            tc.tile_pool(name="small", bufs=4) as smallp,
            tc.tile_pool(name="attn", bufs=8) as attnp,
            tc.tile_pool(name="ytp", bufs=2) as ytp,
            tc.tile_pool(name="pp", bufs=2, space="PSUM") as pp,
            tc.tile_pool(name="scores", bufs=2, space="PSUM") as scp,
            tc.tile_pool(name="accum", bufs=2, space="PSUM") as accp,
            tc.tile_pool(name="dram", bufs=1, space="DRAM") as dram,
        ):
            # ---- constants / weights ----
            ident = constp.tile([128, 128], F32)
            make_identity(nc, ident[:])
            ones_f32 = constp.tile([128, 64], F32)
            nc.vector.memset(ones_f32[:], 1.0)
            ones1 = constp.tile([1, 64], F32R)
            nc.vector.tensor_copy(ones1[:], ones_f32[0:1, :])
            bq_sb = constp.tile([128, 3], F32)
            bo_sb = constp.tile([128, 8], F32)
            wq_sb = constp.tile([128, 8 * RPC], F32R)

            def load_wq(t):
                nc.sync.dma_start(
                    wq_sb[:, t * RPC:(t + 1) * RPC],
                    wqkvT.ap()[t * 128:(t + 1) * 128, :])

            def load_biases():
                nc.sync.dma_start(bq_sb[:], bqkv.ap().rearrange("r p -> p r"))
                nc.sync.dma_start(bo_sb[:], bout.ap().rearrange("t p -> p t"))
            wo_sb = wop.tile([128, 8 * E], F32R)

            def load_wout():
                for t in range(8):
                    nc.sync.dma_start(
                        wo_sb[:, t * E:(t + 1) * E],
                        woutT.ap()[t * 128:(t + 1) * 128, :])

            # persistent per-rep tensors
            qt_sb = qkvp.tile([128, S], F32R, tag="qt")    # [2 heads x 64d, s]
            kt_sb = qkvp.tile([128, S], F32R, tag="kt")
            v_sb = qkvp.tile([128, 2 * NB * (D + 1)], F32R, tag="vsb")
            v_view = v_sb[:].rearrange("p (h b dd) -> p h b dd", h=2, b=NB)
            cc_in = dram.tile([N_CORES, 128, GQ], F32R, tag="ccin")
            cc_out = dram.tile([N_CORES, 128, GQ], F32R, tag="ccout")

            for rep in range(reps):
                # ones columns of V (denominator trick); rewritten each rep
                nc.vector.tensor_copy(
                    v_view[:, :, :, D:D + 1],
                    ones_f32[:].rearrange("p (h b o) -> p h b o", h=2, b=NB, o=1))

                # ---------- proj work-item machinery ----------
                def xt_dmas(g):
                    tiles = []
                    for t in range(8):
                        if g == 0 and rep == 0:
                            load_wq(t)      # interleave weight chunks with first x tiles
                        xt = xtp.tile([128, GQ], F32R, tag="xt")
                        nc.sync.dma_start(
                            xt[:], xT.ap()[t * 128:(t + 1) * 128, g * GQ:(g + 1) * GQ])
                        tiles.append(xt)
                    if g == 0 and rep == 0:
                        load_biases()
                    return tiles

                def proj_items(g, xts):
                    """Yield closures emitting proj instructions for group g."""
                    sl = slice(g * GQ, (g + 1) * GQ)

                    def rtile(which):
                        ps = pp.tile([128, GQ], F32, tag="pp")
                        for t in range(8):
                            lhs = wq_sb[:, t * RPC + which * 128: t * RPC + (which + 1) * 128]
                            yield lambda lhs=lhs, t=t, ps=ps: nc.tensor.matmul(
                                ps[:], lhs, xts[t][:], start=(t == 0), stop=(t == 7))
                        if which == 0:      # q: (psum + bq) * 1/sqrt(D)
                            yield lambda ps=ps: nc.vector.tensor_scalar(
                                qt_sb[:, sl], ps[:], bq_sb[:, 0:1], 0.125, ALU.add, ALU.mult)
                        elif which == 1:    # k: psum + bk
                            yield lambda ps=ps: nc.vector.tensor_scalar(
                                kt_sb[:, sl], ps[:], bq_sb[:, 1:2], None, ALU.add)
                        else:               # v^T staging: psum + bv
                            vt = vstp.tile([128, GQ], F32, tag="vst")
                            yield lambda ps=ps, vt=vt: nc.vector.tensor_scalar(
                                vt[:], ps[:], bq_sb[:, 2:3], None, ALU.add)
                            for j in range(4):
                                bk = 4 * g + j

                                def tr(j=j, bk=bk, vt=vt):
                                    trp = pp.tile([128, GQ], F32, tag="pp")
                                    nc.tensor.transpose(
                                        trp[0:128, 0:128], vt[:, j * 128:(j + 1) * 128],
                                        ident[:])
                                    nc.vector.tensor_copy(
                                        v_view[:, :, bk, 0:D],
                                        trp[0:128, 0:128].rearrange("p (h d) -> p h d", h=2))
                                yield tr
                    yield from rtile(0)
                    yield from rtile(1)
                    yield from rtile(2)

                def attention_group(g, pending):
                    """Emit attention for q-group g, interleaving `pending` proj items."""
                    nbk = 4 * g + 4
                    # throttle interleaved proj items in the first two blocks so the
                    # group's exp pipeline primes before PE picks up filler work
                    quota = []
                    rem = len(pending)
                    for i in range(nbk):
                        if i < 2:
                            q = min(rem, 1)
                        else:
                            left = nbk - i
                            q = (rem + left - 1) // left
                        quota.append(q)
                        rem -= q
                    pt_tiles = {}
                    acc_a = accp.tile([65, GQ], F32, tag="acc")
                    acc_b = accp.tile([65, GQ], F32, tag="acc")
                    for bk in range(nbk):
                        qoff = max(0, (bk - 4 * g)) * 128
                        sc = scp.tile([128, 2 * GQ], F32, tag="sc")
                        nc.tensor.matmul(
                            sc[:, qoff:GQ],
                            kt_sb[0:64, bk * 128:(bk + 1) * 128],
                            qt_sb[0:64, g * GQ + qoff:(g + 1) * GQ],
                            start=True, stop=True, skip_group_check=True)
                        nc.tensor.matmul(
                            sc[:, GQ + qoff:2 * GQ],
                            kt_sb[64:128, bk * 128:(bk + 1) * 128],
                            qt_sb[64:128, g * GQ + qoff:(g + 1) * GQ],
                            start=True, stop=True, skip_group_check=True)
                        pt = ptp.tile([128, 2 * GQ], F32R, tag="pt")
                        nc.scalar.activation(pt[:, qoff:2 * GQ], sc[:, qoff:2 * GQ], ACTF.Exp)
                        pt_tiles[bk] = (pt, qoff)
                        # PV for the previous block (keeps PE busy while ACT exps)
                        if bk > 0:
                            emit_pv(g, bk - 1, pt_tiles, acc_a, acc_b)
                        for _ in range(quota[bk]):
                            if pending:
                                pending.pop(0)()
                    emit_pv(g, nbk - 1, pt_tiles, acc_a, acc_b, last=True)
                    while pending:
                        pending.pop(0)()
                    return normalize_items(g, acc_a, acc_b)

                def normalize_items(g, acc_a, acc_b):
                    # deferred normalize + all-to-all staging closures for group g
                    items = []
                    for h, acc in ((0, acc_a), (1, acc_b)):
                        def norm(h=h, acc=acc, g=g):
                            outu = smallp.tile([65, GQ], F32, tag="outu")
                            nc.vector.tensor_copy(outu[:], acc[:])
                            recip = smallp.tile([1, GQ], F32R, tag="recip")
                            with nc.allow_low_precision(reason="softmax denom reciprocal rounded to fp32r before broadcast"):
                                nc.vector.reciprocal(recip[:], outu[64:65, :])
                            bc = pp.tile([128, GQ], F32, tag="pp")
                            nc.tensor.matmul(bc[0:64, :], ones1[:], recip[:],
                                             start=True, stop=True, skip_group_check=True)
                            bcs = smallp.tile([64, GQ], F32R, tag="bcs")
                            nc.vector.tensor_copy(bcs[:], bc[0:64, :])
                            outn = smallp.tile([64, GQ], F32R, tag="outn")
                            nc.vector.tensor_tensor(outn[:], outu[0:64, :], bcs[:], ALU.mult)
                            nc.gpsimd.dma_start(cc_in[:][g, h * 64:(h + 1) * 64, :], outn[:])
                        items.append(norm)
                    return items

                def emit_pv(g, bk, pt_tiles, acc_a, acc_b, last=False):
                    pt, qoff = pt_tiles.pop(bk)
                    nc.tensor.matmul(
                        acc_a[0:65, qoff:GQ], v_view[:, 0, bk, 0:D + 1], pt[:, qoff:GQ],
                        start=(bk == 0), stop=last, skip_group_check=True)
                    nc.tensor.matmul(
                        acc_b[0:65, qoff:GQ], v_view[:, 1, bk, 0:D + 1],
                        pt[:, GQ + qoff:2 * GQ],
                        start=(bk == 0), stop=last, skip_group_check=True)

                # ---------- emit: proj(0) then attention groups with lookahead ----------
                xts = xt_dmas(0)
                for item in proj_items(0, xts):
                    item()
                carry = []
                for g in range(NG):
                    if g + 1 < NG:
                        nxts = xt_dmas(g + 1)
                        pending = carry + list(proj_items(g + 1, nxts))
                    else:
                        pending = carry
                    carry = attention_group(g, pending)
                    if g == 1:
                        load_wout()
                for item in carry:
                    item()

                # ---------- all-to-all + out_proj ----------
                if cc:
                    nc.gpsimd.collective_compute(
                        "AllToAll", ALU.bypass,
                        replica_groups=[list(range(N_CORES))],
                        ins=[cc_in.opt()], outs=[cc_out.opt()])
                else:
                    nc.gpsimd.dma_start(cc_out[:], cc_in[:])
                at_tiles = []
                for j in range(N_CORES):
                    at = attnp.tile([128, GQ], F32R, tag="at")
                    nc.gpsimd.dma_start(at[:], cc_out[:][j])
                    at_tiles.append(at)
                for t in range(8):
                    ps = pp.tile([128, GQ], F32, tag="pp")
                    for j in range(N_CORES):
                        nc.tensor.matmul(
                            ps[:], wo_sb[:, j * E + t * 128:j * E + (t + 1) * 128],
                            at_tiles[j][:], start=(j == 0), stop=(j == 7))
                    yt = ytp.tile([128, GQ], F32, tag="yt")
                    nc.vector.tensor_scalar(yt[:], ps[:], bo_sb[:, t:t + 1], None, ALU.add)
                    nc.sync.dma_start(yT.ap()[t * 128:(t + 1) * 128, :], yt[:])

    nc.compile()
    return nc


_NC_CACHE = {}


def _get_nc(reps=1):
    if reps not in _NC_CACHE:
        _NC_CACHE[reps] = build_nc(reps)
    return _NC_CACHE[reps]


def make_in_maps(x, in_proj_weight, in_proj_bias, out_proj_weight, out_proj_bias):
    x = np.asarray(x, np.float32)
    w_in = np.asarray(in_proj_weight, np.float32)
    b_in = np.asarray(in_proj_bias, np.float32)
    w_out = np.asarray(out_proj_weight, np.float32)
    b_out = np.asarray(out_proj_bias, np.float32)

    xT = np.ascontiguousarray(x.reshape(S, E).T)
    woutT = np.ascontiguousarray(w_out.T)
    bout = np.ascontiguousarray(b_out.reshape(8, 128))
    in_maps = []
    for c in range(N_CORES):
        rows = []
        for blk in range(3):  # q, k, v blocks of in_proj
            for h in (2 * c, 2 * c + 1):
                rows.extend(range(blk * E + h * D, blk * E + (h + 1) * D))
        rows = np.array(rows)
        wqkvT = np.ascontiguousarray(w_in[rows].T)          # [1024, 384]
        bqkv = np.ascontiguousarray(b_in[rows].reshape(3, 2 * D))
        in_maps.append({
            "xT": xT, "wqkvT": wqkvT, "bqkv": bqkv,
            "woutT": woutT, "bout": bout,
        })
    return in_maps


def assemble_output(results):
    yT_full = np.concatenate([results[c]["yT"] for c in range(N_CORES)], axis=1)
    return np.ascontiguousarray(yT_full.T).reshape(1, S, E).astype(np.float32)


def kernel(x, in_proj_weight, in_proj_bias, out_proj_weight, out_proj_bias,
           block_size, num_heads):
    assert int(np.asarray(block_size)) == BLK and int(np.asarray(num_heads)) == H
    in_maps = make_in_maps(x, in_proj_weight, in_proj_bias,
                           out_proj_weight, out_proj_bias)
    nc = _get_nc(1)
    res = run_bass_kernel_spmd(nc, in_maps, core_ids=list(range(N_CORES)))
    return assemble_output(res.results)
